# revision 1
# baseline (speedup 1.0000x reference)
"""Trainium2 Bass kernel for decomposed relative-position attention (MViT style).

Reference computation (per batch b, head n):
    score = Q K^T / 8  + qterm_h + qterm_w + kterm_h + kterm_w   (L=1024=32x32, C=64)
    out   = softmax(score) V + Q

All four rel-pos bias terms are absorbed into an augmented QK^T matmul:
    Qaug = [Q/8 ; qterm_h^T ; qterm_w^T]          (128 contraction rows)
    Kaug = [K   ; onehot_h(k) ; onehot_w(k)]
    plus a rank-64 second pass  kterm^T x onehot(q)
The score matrix is computed transposed (S^T[k, q]) so that:
  - exp(S^T) tiles are already in lhsT layout for the PV matmul,
  - the softmax denominator comes free as a ones-column appended to V,
  - normalization uses exp(-ln(denom)) broadcast via a K=1 ones-matmul.

Big matmuls run as float32r (full PE rate); the small rel-pos table matmuls
run in bf16 (their magnitude is ~0.1 so the absolute score error is ~1e-3).

Sharding: head-parallel across the 8 NeuronCores (4 batches x 1 head each).
"""

import os
import sys

import numpy as np

if "/opt/trn_rl_repo" not in sys.path:
    sys.path.insert(0, "/opt/trn_rl_repo")

B, NH, L, C = 4, 8, 1024, 64
NCORES = 8

_CACHED = {}


def _build_nc():
    import concourse.bass as bass
    import concourse.tile as tile
    from concourse import bacc, mybir

    f32 = mybir.dt.float32
    f32r = mybir.dt.float32r
    bf16 = mybir.dt.bfloat16
    Exp = mybir.ActivationFunctionType.Exp
    Ln = mybir.ActivationFunctionType.Ln
    Copy = mybir.ActivationFunctionType.Copy
    mult = mybir.AluOpType.mult
    add = mybir.AluOpType.add

    nc = bacc.Bacc("TRN2", target_bir_lowering=False, debug=False)

    import json
    _grp = os.environ.get("KERNEL_GROUPS", "2,2")
    GRP_SIZE = [int(x) for x in _grp.split(",")]
    assert sum(GRP_SIZE) == B
    GRP_START = []
    _acc = 0
    GRP_OF = {}
    for gi, gsz in enumerate(GRP_SIZE):
        GRP_START.append(_acc)
        for pp in range(_acc, _acc + gsz):
            GRP_OF[pp] = gi
        _acc += gsz
    GRP_START_SET = set(GRP_START)

    qts = nc.dram_tensor("qts", [C, B, L], f32, kind="ExternalInput")
    kts = nc.dram_tensor("kts", [C, B, L], f32, kind="ExternalInput")
    qtb = nc.dram_tensor("qtb", [C, B, L], bf16, kind="ExternalInput")
    ktb = nc.dram_tensor("ktb", [C, B, L], bf16, kind="ExternalInput")
    vaug = nc.dram_tensor("vaug", [B, 128, 8, 65], f32, kind="ExternalInput")
    oh_d = nc.dram_tensor("oh", [64, L], f32, kind="ExternalInput")
    tqh_d = nc.dram_tensor("tqh", [64, 63], bf16, kind="ExternalInput")
    tqw_d = nc.dram_tensor("tqw", [64, 63], bf16, kind="ExternalInput")
    tkh_d = nc.dram_tensor("tkh", [64, 63], bf16, kind="ExternalInput")
    tkw_d = nc.dram_tensor("tkw", [64, 63], bf16, kind="ExternalInput")
    ones_d = nc.dram_tensor("ones1", [1, 64], f32, kind="ExternalInput")
    outt = nc.dram_tensor("outt", [B, 64, L], f32, kind="ExternalOutput")
    DEBUG = bool(int(os.environ.get("KERNEL_DEBUG", "0")))
    if DEBUG:
        dbg_s = nc.dram_tensor("dbg_s", [128, L], f32, kind="ExternalOutput")
        dbg_e = nc.dram_tensor("dbg_e", [128, L], f32, kind="ExternalOutput")
        dbg_unn = nc.dram_tensor("dbg_unn", [65, L], f32, kind="ExternalOutput")
        dbg_re = nc.dram_tensor("dbg_re", [64, 512], f32, kind="ExternalOutput")
        dbg_qa = nc.dram_tensor("dbg_qa", [128, L], f32, kind="ExternalOutput")
        dbg_k2 = nc.dram_tensor("dbg_k2", [64, L], f32, kind="ExternalOutput")
        dbg_r1 = nc.dram_tensor("dbg_r1", [1, 512], f32, kind="ExternalOutput")

    with tile.TileContext(nc) as tc:
        with (
            tc.tile_pool(name="consts", bufs=1) as consts,
            tc.tile_pool(name="persist", bufs=1) as persist,
            tc.tile_pool(name="work", bufs=3) as work,
            tc.tile_pool(name="vpool", bufs=4) as vpool,
            tc.tile_pool(name="expp", bufs=8) as expp,
            tc.tile_pool(name="unnp", bufs=2) as unnp,
            tc.tile_pool(name="outp", bufs=2) as outp,
        ):
            # ---- constants (aug-phase dependencies first) ----
            tqh_t = consts.tile([64, 63], bf16)
            nc.sync.dma_start(tqh_t, tqh_d[:])
            tqw_t = consts.tile([64, 63], bf16)
            nc.sync.dma_start(tqw_t, tqw_d[:])
            tkh_t = consts.tile([64, 63], bf16)
            nc.sync.dma_start(tkh_t, tkh_d[:])
            tkw_t = consts.tile([64, 63], bf16)
            nc.sync.dma_start(tkw_t, tkw_d[:])

            # ---- persistent per-core tensors (all 4 pairs resident) ----
            QaugT = persist.tile([128, B, L], f32)
            KaugT = persist.tile([128, B, L], f32)
            KT2 = persist.tile([64, B, L], f32)
            qtb_t = persist.tile([C, B, L], bf16)
            ktb_t = persist.tile([C, B, L], bf16)

            nc.gpsimd.dma_start(qtb_t, qtb[:])
            nc.gpsimd.dma_start(ktb_t, ktb[:])

            oh_t = consts.tile([64, L], f32)
            nc.sync.dma_start(oh_t.bitcast(f32r), oh_d[:].bitcast(f32r))
            ones_t = consts.tile([1, 64], f32)
            nc.sync.dma_start(ones_t, ones_d[:])
            for p in range(B):
                nc.sync.dma_start(QaugT[0:64, p, :].bitcast(f32r), qts[:, p, :].bitcast(f32r))
                nc.sync.dma_start(KaugT[0:64, p, :].bitcast(f32r), kts[:, p, :].bitcast(f32r))
            for p in range(B):
                nc.sync.dma_start(KaugT[64:128, p, :].bitcast(f32r), oh_d[:].bitcast(f32r))

            # ---- rel-pos augmentation terms (bf16), batched across pairs ----
            # For each 4-group chunk t: 4 matmuls of [64,32]^T @ [64, 4x32]
            # into one PSUM bank laid out [g, pair, 32], then one copy out.
            with tc.tile_pool(name="ps_aug", bufs=8, space="PSUM") as ps_aug:
                # (table, src bf16 tile, dst tile, dst row base, w_major)
                terms = [
                    (tqh_t, qtb_t, QaugT, 64, False),
                    (tqw_t, qtb_t, QaugT, 96, True),
                    (tkh_t, ktb_t, KT2, 0, False),
                    (tkw_t, ktb_t, KT2, 32, True),
                ]
                for tbl, src, dst, row0, wmaj in terms:
                    if wmaj:
                        # columns g :: 32  (fixed w=g, h varying)
                        src_r = src.rearrange("c p (h g) -> c p g h", g=32)
                    for t in range(8):
                        ps = ps_aug.tile([32, 512], f32, tag="aug", name="augps")
                        for i in range(4):
                            g = 4 * t + i
                            lhsT = tbl[:, 31 - g: 63 - g]
                            if wmaj:
                                rhs = src_r[:, :, g, :]
                            else:
                                rhs = src[:, :, 32 * g: 32 * g + 32]
                            nc.tensor.matmul(
                                ps[:, 128 * i: 128 * (i + 1)], lhsT, rhs,
                                start=True, stop=True,
                            )
                        # psum layout: [j, (i, pair, x)] with x = w or h
                        src_ap = ps.rearrange("j (i p x) -> j p i x", i=4, p=4)
                        if wmaj:
                            # dst cols q = 32*x + (4t + i)
                            dst_ap = dst[row0:row0 + 32].rearrange(
                                "j p (x g) -> j p g x", g=32
                            )[:, :, 4 * t: 4 * t + 4, :]
                        else:
                            # dst cols q = 32*(4t+i) + x : contiguous 128 block
                            dst_ap = dst[row0:row0 + 32, :, 128 * t: 128 * (t + 1)].rearrange(
                                "j p (i x) -> j p i x", x=32
                            )
                        if t % 2 == 0:
                            nc.vector.tensor_copy(dst_ap.bitcast(f32r), src_ap)
                        else:
                            nc.scalar.activation(dst_ap.bitcast(f32r), src_ap, Copy)

            # ---- main attention loop (all big matmuls in float32r) ----
            unns = []
            pvs = []
            with (
                tc.tile_pool(name="ps_s", bufs=3, space="PSUM") as ps_s,
                tc.tile_pool(name="ps_pv", bufs=1, space="PSUM") as ps_pv,
            ):
                for p in range(B):
                    vg = vpool.tile([128, 8, 65], f32, tag="vg", name="vg")
                    nc.sync.dma_start(vg.bitcast(f32r), vaug[p].bitcast(f32r))

                    pv = ps_pv.tile([65, L], f32, tag="pv", name="pv")
                    pvs.append(pv)
                    for kb in range(8):
                        sp = ps_s.tile([128, L], f32, tag="sp", name="sp")
                        kcols = slice(128 * kb, 128 * (kb + 1))
                        for ch in range(2):
                            cs = slice(512 * ch, 512 * (ch + 1))
                            nc.tensor.matmul(
                                sp[:, cs],
                                KaugT[:, p, kcols].bitcast(f32r),
                                QaugT[:, p, cs].bitcast(f32r),
                                start=True, stop=False,
                            )
                        for ch in range(2):
                            cs = slice(512 * ch, 512 * (ch + 1))
                            nc.tensor.matmul(
                                sp[:, cs],
                                KT2[:, p, kcols].bitcast(f32r),
                                oh_t[:, cs].bitcast(f32r),
                                start=False, stop=True,
                            )
                        et = expp.tile([128, L], f32, tag="et", name="et")
                        if DEBUG and p == 0 and kb == 0:
                            sdmp = work.tile([128, L], f32, tag="sdmp", name="sdmp")
                            nc.vector.tensor_copy(sdmp, sp)
                            nc.sync.dma_start(dbg_s[:], sdmp)
                        nc.scalar.activation(et.bitcast(f32r), sp, Exp)
                        if DEBUG and p == 0 and kb == 0:
                            nc.sync.dma_start(dbg_e[:], et)
                        for ch in range(2):
                            cs = slice(512 * ch, 512 * (ch + 1))
                            nc.tensor.matmul(
                                pv[:, cs],
                                vg[:, kb, :].bitcast(f32r),
                                et[:, cs].bitcast(f32r),
                                start=(kb == 0), stop=(kb == 7),
                            )

                    # free the PSUM accumulator early; normalize below.
                    # The last pair's PSUM is never recycled, so it skips the
                    # SBUF copy and is read directly during normalization.
                    if p < B - 1:
                        unn = unnp.tile([65, L], f32, tag="unn", name="unn")
                        nc.vector.tensor_copy(unn, pv)
                        unns.append(unn)
                    else:
                        unns.append(pv)
                    if DEBUG and p == 0:
                        nc.sync.dma_start(dbg_unn[:], unn)
                        nc.sync.dma_start(dbg_qa[:], QaugT[:, 0, :])
                        nc.sync.dma_start(dbg_k2[:], KT2[:, 0, :])

            # ---- normalization via DVE StreamTranspose + exact reciprocal ----
            # dnb[j, q] = denom[q] (gpsimd broadcast); block-transpose puts
            # denom values on partitions; reciprocal runs on a [64, 32] slice
            # (all 1024 values); broadcast-copy + transpose back yields
            # rS[j, q] = 1/denom[q] with no ACT table switches at all.
            for p in range(B):
                unn = unns[p]
                dnr = work.tile([1, L], f32, tag="dnr", name="dnr", bufs=2)
                nc.vector.tensor_copy(dnr, pvs[p][64:65, :])
                dnb = work.tile([64, L], f32, tag="dnb", name="dnb", bufs=2)
                nc.gpsimd.partition_broadcast(dnb, dnr)
                dnT = work.tile([64, L], f32, tag="dnT", name="dnT", bufs=2)
                nc.vector.transpose(dnT, dnb)
                rT = work.tile([64, 32], f32, tag="rT", name="rT", bufs=2)
                nc.vector.reciprocal(
                    rT, dnT.rearrange("i (c j) -> i j c", j=32)[:, 0, :]
                )
                rE = work.tile([64, 32, 32], f32, tag="rE", name="rE", bufs=2)
                nc.vector.tensor_copy(
                    rE, rT[:, :, None].to_broadcast((64, 32, 32))
                )
                rS = work.tile([64, L], f32, tag="rS", name="rS", bufs=2)
                nc.vector.transpose(rS, rE.rearrange("i c j -> i (c j)"))
                if DEBUG and p == 0:
                    nc.sync.dma_start(dbg_r1[:], rS[0:1, 0:512])
                    nc.sync.dma_start(dbg_re[:], rS[:, 0:512])
                ot = outp.tile([64, L], f32, tag="ot", name="ot")
                for ch in range(2):
                    cs = slice(512 * ch, 512 * (ch + 1))
                    tmp = work.tile([64, 512], f32, tag="tmp", name="tmp")
                    nc.vector.tensor_mul(tmp, unn[0:64, cs], rS[:, cs])
                    nc.vector.scalar_tensor_tensor(
                        ot[:, cs], QaugT[0:64, p, cs], 8.0, tmp, mult, add
                    )
                    nc.sync.dma_start(outt[p, :, cs], ot[:, cs])

    nc.compile()
    return nc


def _host_consts():
    oh = np.zeros((64, L), np.float32)
    qq = np.arange(L)
    oh[qq // 32, qq] = 1.0
    oh[32 + qq % 32, qq] = 1.0
    ones1 = np.ones((1, 64), np.float32)
    return oh, ones1


def kernel(query, key_input, value, rel_h_q, rel_w_q, rel_h_k, rel_w_k):
    from concourse.bass_utils import run_bass_kernel_spmd

    query = np.asarray(query, np.float32)
    key_input = np.asarray(key_input, np.float32)
    value = np.asarray(value, np.float32)
    rel_h_q = np.asarray(rel_h_q, np.float32)
    rel_w_q = np.asarray(rel_w_q, np.float32)
    rel_h_k = np.asarray(rel_h_k, np.float32)
    rel_w_k = np.asarray(rel_w_k, np.float32)

    if "nc" not in _CACHED:
        _CACHED["nc"] = _build_nc()
    nc = _CACHED["nc"]

    import ml_dtypes

    bf = ml_dtypes.bfloat16
    oh, ones1 = _host_consts()
    tqh = np.ascontiguousarray(rel_h_q[::-1].T).astype(bf)
    tqw = np.ascontiguousarray(rel_w_q[::-1].T).astype(bf)
    tkh = np.ascontiguousarray(rel_h_k.T).astype(bf)
    tkw = np.ascontiguousarray(rel_w_k.T).astype(bf)

    in_maps = []
    for n in range(NCORES):
        qt = np.ascontiguousarray(query[:, n].transpose(2, 0, 1))
        kt = np.ascontiguousarray(key_input[:, n].transpose(2, 0, 1))
        v = value[:, n]
        va = np.concatenate([v, np.ones((B, L, 1), np.float32)], -1)
        va = np.ascontiguousarray(va.reshape(B, 8, 128, 65).transpose(0, 2, 1, 3))
        in_maps.append(
            dict(qts=qt * 0.125, kts=kt, qtb=qt.astype(bf), ktb=kt.astype(bf),
                 vaug=va, oh=oh, tqh=tqh, tqw=tqw, tkh=tkh, tkw=tkw, ones1=ones1)
        )

    res = run_bass_kernel_spmd(
        nc, in_maps, core_ids=list(range(NCORES)),
        trace=bool(int(os.environ.get("KERNEL_TRACE", "0"))),
    )
    _CACHED["last_result"] = res

    out = np.stack([r["outt"] for r in res.results], axis=1)  # [B, NH, 64, L]
    return np.ascontiguousarray(out.transpose(0, 1, 3, 2)).astype(np.float32)



# revision 2
# speedup vs baseline: 1.0078x; 1.0078x over previous
"""Trainium2 Bass kernel v2: decomposed rel-pos attention via fp8 DoubleRow.

Score factorization (per batch p, head n), computed transposed S^T[k, q] in
ONE fp8 DoubleRow matmul (256-row contraction, 0.5 cycles/row):
      pair0 rows 0:64   K^T[c,k]       x  Q^T[c,q]/8
      pair0 rows 64:96  onehot_h(k)    x  qterm_h^T[j,q]
      pair0 rows 96:128 onehot_w(k)    x  qterm_w^T[j,q]
      pair1 rows 0:32   kterm_h^T[j,k] x  onehot_h(q)
      pair1 rows 32:64  kterm_w^T[j,k] x  onehot_w(q)
      pair1 rows 64:128 zeros
    et = exp(S^T - 4)        (ACT engine is the bottleneck at ~34us;
                              -4 keeps fp8 range safe, cancels in softmax)
    out^T[q, c] = (et^T V) / (et^T 1) + Q    (PV emitted in [q, c] layout:
                              65-wide free dim makes it nearly free on PE;
                              denominator lands per-partition so normalize
                              is reciprocal + 2 vector ops, no transposes)

qterm/kterm come from 32-wide fp8 table matmuls grouped by h/w of q/k into
one [128, 1024] PSUM tile (4 partition strips: qh, qw, kh, kw), then two
copies (DVE q-side, Pool k-side) drop them into the operand tiles.
q-side tables are pre-scaled x8 so the matmuls read Q/8 in place.

PSUM start_tensor_calc zeroes lazily per (partition-range x 2KB bank): only
the first matmul touching a bank (per partition strip) carries start=True;
w-strips straddle both banks so zero-writing armer matmuls arm them.

Per-batch operand tiles keep DMA/copy/read dependencies exact, and S
matmuls are emitted one k-pair ahead of PV so the in-order PE queue never
head-of-line blocks the exp pipeline.

Sharding: head-parallel across the 8 NeuronCores (4 batches x 1 head each).
"""

import os
import sys

import numpy as np

if "/opt/trn_rl_repo" not in sys.path:
    sys.path.insert(0, "/opt/trn_rl_repo")

B, NH, L, C = 4, 8, 1024, 64
NCORES = 8

_CACHED = {}


def _build_nc():
    import concourse.bass as bass  # noqa: F401
    import concourse.tile as tile
    from concourse import bacc, mybir

    f32 = mybir.dt.float32
    fp8 = mybir.dt.float8e4
    Exp = mybir.ActivationFunctionType.Exp
    DR = mybir.MatmulPerfMode.DoubleRow

    nc = bacc.Bacc("TRN2", target_bir_lowering=False, debug=False)

    qfa_d = nc.dram_tensor("qfa", [B, 64, L], fp8, kind="ExternalInput")
    qfb_d = nc.dram_tensor("qfb", [B, 128, L], fp8, kind="ExternalInput")
    kfa_d = nc.dram_tensor("kfa", [B, 128, L], fp8, kind="ExternalInput")
    kfb_d = nc.dram_tensor("kfb", [B, 64, L], fp8, kind="ExternalInput")
    const8_d = nc.dram_tensor("const8", [128, 512], fp8, kind="ExternalInput")
    v8_d = nc.dram_tensor("v8", [B, 128, 4, 2, 64], fp8, kind="ExternalInput")
    qres_d = nc.dram_tensor("qres", [128, 8, B, 64], f32, kind="ExternalInput")
    outt = nc.dram_tensor("outt", [B, 128, 8, 64], f32, kind="ExternalOutput")

    with tile.TileContext(nc) as tc:
        with (
            tc.tile_pool(name="persist", bufs=1) as persist,
            tc.tile_pool(name="work", bufs=2) as work,
            tc.tile_pool(name="expp", bufs=3) as expp,
            tc.tile_pool(name="outp", bufs=2) as outp,
        ):
            biasc = persist.tile([128, 1], f32)
            nc.gpsimd.memset(biasc, -4.0)
            z64a = persist.tile([64, 32], fp8)
            nc.gpsimd.memset(z64a, 0.0)
            warm = persist.tile([128, 1], f32)
            nc.scalar.activation(warm, biasc, Exp, bias=biasc)

            const_t = persist.tile([128, 512], fp8)
            nc.sync.dma_start(const_t, const8_d[:])
            tbl = const_t[0:64, 0:252].rearrange("c (t m) -> c t m", t=4)
            ones2 = const_t[:, 504:506].rearrange("p (i o) -> p i o", o=1)

            # per-batch operand tiles; batch-0 pieces issued first
            Qf = [persist.tile([128, 2, L], fp8, name=f"Qf{p}") for p in range(B)]
            Kf = [persist.tile([128, 2, L], fp8, name=f"Kf{p}") for p in range(B)]
            v8t = [persist.tile([128, 4, 2, 64], fp8, name=f"v8{p}") for p in range(B)]
            def load_batch(p, eng):
                # aug regions Qf[64:128, 0] / Kf[0:64, 1] are device-written;
                # batch 0's aug inputs ride SWDGE to dodge the serial HWDGE
                eng.dma_start(Qf[p][0:64, 0, :], qfa_d[p])
                eng.dma_start(Kf[p][:, 0, :], kfa_d[p])
                beng = nc.scalar if p == 0 else nc.sync
                beng.dma_start(Qf[p][:, 1, :], qfb_d[p])
                beng.dma_start(Kf[p][64:128, 1, :], kfb_d[p])
                nc.scalar.dma_start(v8t[p], v8_d[p])

            load_batch(0, nc.gpsimd)
            for p in range(1, B):
                load_batch(p, nc.sync)
            qres_t = persist.tile([128, 8, B, 64], f32)
            nc.sync.dma_start(qres_t, qres_d[:])

            with (
                tc.tile_pool(name="ps_aug", bufs=1, space="PSUM") as ps_aug,
                tc.tile_pool(name="ps_s", bufs=2, space="PSUM") as ps_s,
                tc.tile_pool(name="ps_pv", bufs=1, space="PSUM") as ps_pv,
                tc.tile_pool(name="ps_den", bufs=1, space="PSUM") as ps_den,
            ):
                def emit_aug_strips(ps_a, ps_aw, p, ts):
                    for t in ts:
                        src = (Qf[p] if t < 2 else Kf[p])[0:64, 0, :]
                        src_w = src.rearrange("c (h w) -> c w h", w=32)
                        if t % 2 == 1:
                            for bank in range(2):
                                nc.tensor.matmul(
                                    ps_a[32 * t:32 * t + 32,
                                         512 * bank:512 * bank + 1],
                                    z64a, const_t[0:64, 0:1],
                                    start=True, stop=True,
                                    tile_position=(0, 32 * t),
                                    skip_group_check=True)
                        for g in range(32):
                            lhsT = tbl[:, t, 31 - g:63 - g]
                            if t % 2 == 0:
                                rhs = src[:, 32 * g:32 * g + 32]
                                out = ps_a[32 * t:32 * t + 32, 32 * g:32 * g + 32]
                                st = g in (0, 16)
                            else:
                                rhs = src_w[:, g, :]
                                out = ps_aw[32 * t:32 * t + 32, g, :]
                                st = False
                            nc.tensor.matmul(out, lhsT, rhs, start=st, stop=True,
                                             tile_position=(0, 32 * t),
                                             skip_group_check=True)

                aug_tiles = {}

                def emit_aug_step(p, step):
                    if step == 0:
                        ps_a = ps_aug.tile([128, L], f32, tag="psa", name="psa")
                        aug_tiles[p] = (ps_a, ps_a.rearrange("j (h w) -> j w h",
                                                             w=32))
                    ps_a, ps_aw = aug_tiles[p]
                    emit_aug_strips(ps_a, ps_aw, p, (step,))
                    if step == 1:
                        nc.vector.tensor_copy(Qf[p][64:128, 0, 0:512],
                                              ps_a[0:64, 0:512])
                        nc.vector.tensor_copy(Qf[p][64:128, 0, 512:L],
                                              ps_a[0:64, 512:L])
                    elif step == 3:
                        nc.vector.tensor_copy(Kf[p][0:64, 1, 0:128],
                                              ps_a[64:128, 0:128])
                        nc.vector.tensor_copy(Kf[p][0:64, 1, 128:L],
                                              ps_a[64:128, 128:L])

                def emit_aug(p):
                    for step in range(4):
                        emit_aug_step(p, step)

                emit_aug(0)

                state = {}

                def emit_s_exp(p, kb):
                    st = state[p]
                    if kb % 2 == 0:
                        st["et2"].append(expp.tile([128, 2, L], fp8,
                                                   tag="et2", name="et2"))
                    et2 = st["et2"][kb // 2]
                    sp = ps_s.tile([128, L], f32, tag="sp", name="sp")
                    for ch in range(2):
                        cs = slice(512 * ch, 512 * (ch + 1))
                        nc.tensor.matmul(
                            sp[:, cs],
                            Kf[p][:, :, 128 * kb:128 * (kb + 1)],
                            Qf[p][:, :, cs],
                            start=True, stop=True, perf_mode=DR)
                    nc.scalar.activation(et2[:, kb % 2, :], sp, Exp, bias=biasc)

                def emit_pv(p, kbp):
                    st = state[p]
                    et2 = st["et2"][kbp]
                    for qb in range(8):
                        lhsT = et2[:, :, 128 * qb:128 * (qb + 1)]
                        first = kbp == 0 and qb == 0
                        nc.tensor.matmul(st["pvt"][:, qb, :], lhsT,
                                         v8t[p][:, kbp, :, :],
                                         start=first, stop=(kbp == 3),
                                         perf_mode=DR, skip_group_check=True)
                        nc.tensor.matmul(st["den"][:, qb:qb + 1], lhsT, ones2,
                                         start=first, stop=(kbp == 3),
                                         perf_mode=DR, skip_group_check=True)

                def emit_norm(p, quarters=False):
                    # normalize + residual in qb-chunks so each output DMA
                    # overlaps the next chunk's vector work
                    st = state[p]
                    r = work.tile([128, 8], f32, tag="r", name="r")
                    nc.vector.reciprocal(r, st["den"])
                    ot = outp.tile([128, 8, 64], f32, tag="ot", name="ot")
                    nch = 4 if quarters else 2
                    w_ = 8 // nch
                    for hb in range(nch):
                        hs = slice(w_ * hb, w_ * hb + w_)
                        otm = work.tile([128, w_, 64], f32, tag="otm",
                                        name="otm", bufs=2)
                        nc.vector.tensor_mul(
                            otm, st["pvt"][:, hs, :],
                            r[:, hs, None].to_broadcast((128, w_, 64)))
                        # adds are SBUF-only so Pool may take half of them
                        (nc.gpsimd if hb % 2 else nc.vector).tensor_add(
                            ot[:, hs, :], otm, qres_t[:, hs, p, :])
                        nc.sync.dma_start(outt[p, :, hs, :], ot[:, hs, :])

                # software pipeline: PV(p, m) emits three S/exp slots
                # after exp(p, 2m+1) and the next batch's aug strips spread
                # over kb 2..5, so the in-order PE queue never blocks the
                # exp stream; normalize(p) slides into batch p+1
                stream = [(p, kb) for p in range(B) for kb in range(8)]
                for idx, (p, kb) in enumerate(stream):
                    if kb == 0:
                        state[p] = dict(
                            pvt=ps_pv.tile([128, 8, 64], f32, tag="pvt",
                                           name="pvt"),
                            den=ps_den.tile([128, 8], f32, tag="den",
                                            name="den"),
                            et2=[])
                    emit_s_exp(p, kb)
                    aug0 = 2
                    if aug0 <= kb <= aug0 + 3 and p + 1 < B:
                        emit_aug_step(p + 1, kb - aug0)
                    due = idx - 3
                    if due >= 0:
                        dp, dkb = stream[due]
                        if dkb % 2 == 1:
                            emit_pv(dp, dkb // 2)
                            if dkb == 7:
                                emit_norm(dp)
                # flush whatever the lag left pending
                lag = 3
                for due in range(len(stream) - lag, len(stream)):
                    dp, dkb = stream[due]
                    if dkb % 2 == 1:
                        emit_pv(dp, dkb // 2)
                        if dkb == 7:
                            emit_norm(dp, quarters=(dp == B - 1))

    nc.compile()
    return nc


def _split_c(x):
    # [64, ...] -> [32, 2, ...] with c = 32*i + ci
    return np.ascontiguousarray(
        x.reshape(2, 32, *x.shape[1:]).transpose(1, 0, *range(2, x.ndim + 1)))


def kernel(query, key_input, value, rel_h_q, rel_w_q, rel_h_k, rel_w_k):
    import ml_dtypes
    from concourse.bass_utils import run_bass_kernel_spmd

    f8 = ml_dtypes.float8_e4m3
    query = np.asarray(query, np.float32)
    key_input = np.asarray(key_input, np.float32)
    value = np.asarray(value, np.float32)

    if "nc" not in _CACHED:
        _CACHED["nc"] = _build_nc()
    nc = _CACHED["nc"]

    ll = np.arange(L)
    oh_h = (ll // 32 == np.arange(32)[:, None]).astype(np.float32)  # [32, L]
    oh_w = (ll % 32 == np.arange(32)[:, None]).astype(np.float32)

    # tables [4(t), 64(c), 63(m)] -> const8 rows 0:64; q-side tables x8
    tables = np.stack([
        np.asarray(rel_h_q, np.float32)[::-1].T * 8.0,
        np.asarray(rel_w_q, np.float32)[::-1].T * 8.0,
        np.asarray(rel_h_k, np.float32).T,
        np.asarray(rel_w_k, np.float32).T,
    ], 0)
    const8 = np.zeros((128, 512), np.float32)
    const8[0:64, 0:252] = tables.transpose(1, 0, 2).reshape(64, 252)
    const8[:, 504:506] = 1.0
    const8 = const8.astype(f8)

    z64 = np.zeros((64, L), np.float32)

    in_maps = []
    for n in range(NCORES):
        q = query[:, n]           # [B, L, C]
        k = key_input[:, n]
        v = value[:, n]
        qT = q.transpose(2, 0, 1)  # [C, B, L]
        kT = k.transpose(2, 0, 1)
        # qf[p]: [128, 2, L]: pair0 = [Q^T/8 ; qterm placeholder]
        #                     pair1 = [onehot_h(q); onehot_w(q); zeros]
        qfa = np.ascontiguousarray(qT.transpose(1, 0, 2) / 8.0).astype(f8)
        qfb1 = np.concatenate([oh_h, oh_w, z64], 0)  # [128, L]
        qfb = np.ascontiguousarray(
            np.broadcast_to(qfb1[None], (B, 128, L))).astype(f8)
        kfa = np.ascontiguousarray(np.concatenate(
            [kT.transpose(1, 0, 2),
             np.broadcast_to(oh_h[None], (B, 32, L)),
             np.broadcast_to(oh_w[None], (B, 32, L))], 1)).astype(f8)
        kfb = np.zeros((B, 64, L), f8)
        # v8[p]: [128, 4(kbp), 2(i), 64]; k = (2*kbp + i)*128 + kp
        v8 = np.ascontiguousarray(
            v.reshape(B, 4, 2, 128, 64).transpose(0, 3, 1, 2, 4)).astype(f8)
        qres = np.ascontiguousarray(
            q.reshape(B, 8, 128, 64).transpose(2, 1, 0, 3)).astype(np.float32)
        in_maps.append(dict(qfa=qfa, qfb=qfb, kfa=kfa, kfb=kfb,
                            const8=const8, v8=v8, qres=qres))

    res = run_bass_kernel_spmd(
        nc, in_maps, core_ids=list(range(NCORES)),
        trace=bool(int(os.environ.get("KERNEL_TRACE", "0"))),
    )
    _CACHED["last_result"] = res

    # outt: [B, 128, 8, 64] -> out[b, n, qb*128+qp, c]
    out = np.stack([r["outt"] for r in res.results], axis=1)  # [B, NH, 128, 8, 64]
    out = out.transpose(0, 1, 3, 2, 4).reshape(B, NH, L, C)
    return np.ascontiguousarray(out).astype(np.float32)


# revision 3
# speedup vs baseline: 1.0152x; 1.0074x over previous
"""Trainium2 Bass kernel v2: decomposed rel-pos attention via fp8 DoubleRow.

Score factorization (per batch p, head n), computed transposed S^T[k, q] in
ONE fp8 DoubleRow matmul (256-row contraction, 0.5 cycles/row):
      pair0 rows 0:64   K^T[c,k]       x  Q^T[c,q]/8
      pair0 rows 64:96  onehot_h(k)    x  qterm_h^T[j,q]
      pair0 rows 96:128 onehot_w(k)    x  qterm_w^T[j,q]
      pair1 rows 0:32   kterm_h^T[j,k] x  onehot_h(q)
      pair1 rows 32:64  kterm_w^T[j,k] x  onehot_w(q)
      pair1 rows 64:128 zeros
    et = exp(S^T - 4)        (ACT engine is the bottleneck at ~34us;
                              -4 keeps fp8 range safe, cancels in softmax)
    out^T[q, c] = (et^T V) / (et^T 1) + Q    (PV emitted in [q, c] layout:
                              65-wide free dim makes it nearly free on PE;
                              denominator lands per-partition so normalize
                              is reciprocal + 2 vector ops, no transposes)

qterm/kterm come from 32-wide fp8 table matmuls grouped by h/w of q/k into
one [128, 1024] PSUM tile (4 partition strips: qh, qw, kh, kw), then two
copies (DVE q-side, Pool k-side) drop them into the operand tiles.
q-side tables are pre-scaled x8 so the matmuls read Q/8 in place.

PSUM start_tensor_calc zeroes lazily per (partition-range x 2KB bank): only
the first matmul touching a bank (per partition strip) carries start=True;
w-strips straddle both banks so zero-writing armer matmuls arm them.

Per-batch operand tiles keep DMA/copy/read dependencies exact, and S
matmuls are emitted one k-pair ahead of PV so the in-order PE queue never
head-of-line blocks the exp pipeline.

Sharding: head-parallel across the 8 NeuronCores (4 batches x 1 head each).
"""

import os
import sys

import numpy as np

if "/opt/trn_rl_repo" not in sys.path:
    sys.path.insert(0, "/opt/trn_rl_repo")

B, NH, L, C = 4, 8, 1024, 64
NCORES = 8

_CACHED = {}


def _build_nc():
    import concourse.bass as bass  # noqa: F401
    import concourse.tile as tile
    from concourse import bacc, mybir

    f32 = mybir.dt.float32
    fp8 = mybir.dt.float8e4
    Exp = mybir.ActivationFunctionType.Exp
    DR = mybir.MatmulPerfMode.DoubleRow

    nc = bacc.Bacc("TRN2", target_bir_lowering=False, debug=False)

    qfa_d = nc.dram_tensor("qfa", [B, 64, L], fp8, kind="ExternalInput")
    qfb_d = nc.dram_tensor("qfb", [B, 128, L], fp8, kind="ExternalInput")
    kfa_d = nc.dram_tensor("kfa", [B, 128, L], fp8, kind="ExternalInput")
    kfb_d = nc.dram_tensor("kfb", [B, 64, L], fp8, kind="ExternalInput")
    const8_d = nc.dram_tensor("const8", [128, 512], fp8, kind="ExternalInput")
    v8_d = nc.dram_tensor("v8", [B, 128, 4, 2, 64], fp8, kind="ExternalInput")
    qres_d = nc.dram_tensor("qres", [128, 8, B, 64], f32, kind="ExternalInput")
    outt = nc.dram_tensor("outt", [B, 128, 8, 64], f32, kind="ExternalOutput")

    with tile.TileContext(nc) as tc:
        with (
            tc.tile_pool(name="persist", bufs=1) as persist,
            tc.tile_pool(name="work", bufs=2) as work,
            tc.tile_pool(name="expp", bufs=3) as expp,
            tc.tile_pool(name="outp", bufs=2) as outp,
        ):
            biasc = persist.tile([128, 1], f32)
            nc.gpsimd.memset(biasc, -4.0)
            z64a = persist.tile([64, 32], fp8)
            nc.gpsimd.memset(z64a, 0.0)
            warm = persist.tile([128, 1], f32)
            nc.scalar.activation(warm, biasc, Exp, bias=biasc)

            const_t = persist.tile([128, 512], fp8)
            nc.sync.dma_start(const_t, const8_d[:])
            tbl = const_t[0:64, 0:252].rearrange("c (t m) -> c t m", t=4)
            ones2 = const_t[:, 504:506].rearrange("p (i o) -> p i o", o=1)

            # per-batch operand tiles; batch-0 pieces issued first
            Qf = [persist.tile([128, 2, L], fp8, name=f"Qf{p}") for p in range(B)]
            Kf = [persist.tile([128, 2, L], fp8, name=f"Kf{p}") for p in range(B)]
            v8t = [persist.tile([128, 4, 2, 64], fp8, name=f"v8{p}") for p in range(B)]
            def load_batch(p, eng):
                # aug regions Qf[64:128, 0] / Kf[0:64, 1] are device-written;
                # batch 0's aug inputs ride SWDGE to dodge the serial HWDGE
                eng.dma_start(Qf[p][0:64, 0, :], qfa_d[p])
                eng.dma_start(Kf[p][:, 0, :], kfa_d[p])
                beng = nc.scalar if p == 0 else nc.sync
                beng.dma_start(Qf[p][:, 1, :], qfb_d[p])
                beng.dma_start(Kf[p][64:128, 1, :], kfb_d[p])
                nc.scalar.dma_start(v8t[p], v8_d[p])

            load_batch(0, nc.gpsimd)
            for p in range(1, B):
                load_batch(p, nc.sync)
            qres_t = persist.tile([128, 8, B, 64], f32)
            nc.sync.dma_start(qres_t, qres_d[:])

            # prologue aug(0) in two separate 2-bank tiles (q-side, k-side):
            # byte-interval dependency tracking is partition-blind, so a
            # shared tile would serialize k-strips behind q-copies; the
            # scoped pool frees its banks before the main pools open
            with tc.tile_pool(name="ps_aug0", bufs=2, space="PSUM") as ps_aug0:
                tq0 = ps_aug0.tile([64, L], f32, name="tq0")
                tk0 = ps_aug0.tile([64, L], f32, name="tk0")
                for t in range(4):
                    tile_, row0 = (tq0, 32 * t) if t < 2 else (tk0, 32 * (t - 2))
                    tile_w = tile_.rearrange("j (h w) -> j w h", w=32)
                    src = (Qf[0] if t < 2 else Kf[0])[0:64, 0, :]
                    src_w = src.rearrange("c (h w) -> c w h", w=32)
                    if t % 2 == 1:
                        for bank in range(2):
                            nc.tensor.matmul(
                                tile_[row0:row0 + 32,
                                      512 * bank:512 * bank + 1],
                                z64a, const_t[0:64, 0:1],
                                start=True, stop=True,
                                tile_position=(0, row0),
                                skip_group_check=True)
                    for g in range(32):
                        lhsT = tbl[:, t, 31 - g:63 - g]
                        if t % 2 == 0:
                            rhs = src[:, 32 * g:32 * g + 32]
                            out = tile_[row0:row0 + 32, 32 * g:32 * g + 32]
                            st = g in (0, 16)
                        else:
                            rhs = src_w[:, g, :]
                            out = tile_w[row0:row0 + 32, g, :]
                            st = False
                        nc.tensor.matmul(out, lhsT, rhs, start=st, stop=True,
                                         tile_position=(0, row0),
                                         skip_group_check=True)
                    if t == 1:
                        nc.vector.tensor_copy(Qf[0][64:128, 0, 0:512],
                                              tq0[:, 0:512])
                        nc.vector.tensor_copy(Qf[0][64:128, 0, 512:L],
                                              tq0[:, 512:L])
                    elif t == 3:
                        nc.vector.tensor_copy(Kf[0][0:64, 1, 0:512],
                                              tk0[:, 0:512])
                        nc.vector.tensor_copy(Kf[0][0:64, 1, 512:L],
                                              tk0[:, 512:L])

            with (
                tc.tile_pool(name="ps_aug", bufs=1, space="PSUM") as ps_aug,
                tc.tile_pool(name="ps_s", bufs=2, space="PSUM") as ps_s,
                tc.tile_pool(name="ps_pv", bufs=1, space="PSUM") as ps_pv,
                tc.tile_pool(name="ps_den", bufs=1, space="PSUM") as ps_den,
            ):
                def emit_aug_strips(ps_a, ps_aw, p, ts):
                    for t in ts:
                        src = (Qf[p] if t < 2 else Kf[p])[0:64, 0, :]
                        src_w = src.rearrange("c (h w) -> c w h", w=32)
                        if t % 2 == 1:
                            for bank in range(2):
                                nc.tensor.matmul(
                                    ps_a[32 * t:32 * t + 32,
                                         512 * bank:512 * bank + 1],
                                    z64a, const_t[0:64, 0:1],
                                    start=True, stop=True,
                                    tile_position=(0, 32 * t),
                                    skip_group_check=True)
                        for g in range(32):
                            lhsT = tbl[:, t, 31 - g:63 - g]
                            if t % 2 == 0:
                                rhs = src[:, 32 * g:32 * g + 32]
                                out = ps_a[32 * t:32 * t + 32, 32 * g:32 * g + 32]
                                st = g in (0, 16)
                            else:
                                rhs = src_w[:, g, :]
                                out = ps_aw[32 * t:32 * t + 32, g, :]
                                st = False
                            nc.tensor.matmul(out, lhsT, rhs, start=st, stop=True,
                                             tile_position=(0, 32 * t),
                                             skip_group_check=True)

                aug_tiles = {}

                def emit_aug_step(p, step, act_assist=False):
                    if step == 0:
                        ps_a = ps_aug.tile([128, L], f32, tag="psa", name="psa")
                        aug_tiles[p] = (ps_a, ps_a.rearrange("j (h w) -> j w h",
                                                             w=32))
                    ps_a, ps_aw = aug_tiles[p]
                    emit_aug_strips(ps_a, ps_aw, p, (step,))
                    # ACT is idle before the first exp, so the prologue
                    # parallelizes the PSUM->SBUF copies across DVE + ACT
                    if step == 1:
                        nc.vector.tensor_copy(Qf[p][64:128, 0, 0:512],
                                              ps_a[0:64, 0:512])
                        (nc.scalar.copy if act_assist
                         else nc.vector.tensor_copy)(
                            Qf[p][64:128, 0, 512:L], ps_a[0:64, 512:L])
                    elif step == 3:
                        nc.vector.tensor_copy(Kf[p][0:64, 1, 0:128],
                                              ps_a[64:128, 0:128])
                        (nc.scalar.copy if act_assist
                         else nc.vector.tensor_copy)(
                            Kf[p][0:64, 1, 128:L], ps_a[64:128, 128:L])

                def emit_aug(p):
                    for step in range(4):
                        emit_aug_step(p, step)

                state = {}

                def emit_s_exp(p, kb):
                    st = state[p]
                    if kb % 2 == 0:
                        st["et2"].append(expp.tile([128, 2, L], fp8,
                                                   tag="et2", name="et2"))
                    et2 = st["et2"][kb // 2]
                    sp = ps_s.tile([128, L], f32, tag="sp", name="sp")
                    for ch in range(2):
                        cs = slice(512 * ch, 512 * (ch + 1))
                        nc.tensor.matmul(
                            sp[:, cs],
                            Kf[p][:, :, 128 * kb:128 * (kb + 1)],
                            Qf[p][:, :, cs],
                            start=True, stop=True, perf_mode=DR)
                    nc.scalar.activation(et2[:, kb % 2, :], sp, Exp, bias=biasc)

                def emit_pv(p, kbp):
                    st = state[p]
                    et2 = st["et2"][kbp]
                    for qb in range(8):
                        lhsT = et2[:, :, 128 * qb:128 * (qb + 1)]
                        first = kbp == 0 and qb == 0
                        nc.tensor.matmul(st["pvt"][:, qb, :], lhsT,
                                         v8t[p][:, kbp, :, :],
                                         start=first, stop=(kbp == 3),
                                         perf_mode=DR, skip_group_check=True)
                        nc.tensor.matmul(st["den"][:, qb:qb + 1], lhsT, ones2,
                                         start=first, stop=(kbp == 3),
                                         perf_mode=DR, skip_group_check=True)

                def emit_norm(p, quarters=False):
                    # normalize + residual in qb-chunks so each output DMA
                    # overlaps the next chunk's vector work
                    st = state[p]
                    r = work.tile([128, 8], f32, tag="r", name="r")
                    nc.vector.reciprocal(r, st["den"])
                    ot = outp.tile([128, 8, 64], f32, tag="ot", name="ot")
                    nch = 4 if quarters else 2
                    w_ = 8 // nch
                    for hb in range(nch):
                        hs = slice(w_ * hb, w_ * hb + w_)
                        otm = work.tile([128, w_, 64], f32, tag="otm",
                                        name="otm", bufs=2)
                        nc.vector.tensor_mul(
                            otm, st["pvt"][:, hs, :],
                            r[:, hs, None].to_broadcast((128, w_, 64)))
                        (nc.gpsimd if hb % 2 else nc.vector).tensor_add(
                            ot[:, hs, :], otm, qres_t[:, hs, p, :])
                        nc.sync.dma_start(outt[p, :, hs, :], ot[:, hs, :])

                # software pipeline: PV(p, m) emits three S/exp slots
                # after exp(p, 2m+1) and the next batch's aug strips spread
                # over kb 2..5, so the in-order PE queue never blocks the
                # exp stream; normalize(p) slides into batch p+1
                stream = [(p, kb) for p in range(B) for kb in range(8)]
                for idx, (p, kb) in enumerate(stream):
                    if kb == 0:
                        state[p] = dict(
                            pvt=ps_pv.tile([128, 8, 64], f32, tag="pvt",
                                           name="pvt"),
                            den=ps_den.tile([128, 8], f32, tag="den",
                                            name="den"),
                            et2=[])
                    emit_s_exp(p, kb)
                    aug0 = 2
                    if aug0 <= kb <= aug0 + 3 and p + 1 < B:
                        emit_aug_step(p + 1, kb - aug0)
                    due = idx - 3
                    if due >= 0:
                        dp, dkb = stream[due]
                        if dkb % 2 == 1:
                            emit_pv(dp, dkb // 2)
                            if dkb == 7:
                                emit_norm(dp)
                # flush whatever the lag left pending
                lag = 3
                for due in range(len(stream) - lag, len(stream)):
                    dp, dkb = stream[due]
                    if dkb % 2 == 1:
                        emit_pv(dp, dkb // 2)
                        if dkb == 7:
                            emit_norm(dp, quarters=(dp == B - 1))

    nc.compile()
    return nc


def _split_c(x):
    # [64, ...] -> [32, 2, ...] with c = 32*i + ci
    return np.ascontiguousarray(
        x.reshape(2, 32, *x.shape[1:]).transpose(1, 0, *range(2, x.ndim + 1)))


def kernel(query, key_input, value, rel_h_q, rel_w_q, rel_h_k, rel_w_k):
    import ml_dtypes
    from concourse.bass_utils import run_bass_kernel_spmd

    f8 = ml_dtypes.float8_e4m3
    query = np.asarray(query, np.float32)
    key_input = np.asarray(key_input, np.float32)
    value = np.asarray(value, np.float32)

    if "nc" not in _CACHED:
        _CACHED["nc"] = _build_nc()
    nc = _CACHED["nc"]

    ll = np.arange(L)
    oh_h = (ll // 32 == np.arange(32)[:, None]).astype(np.float32)  # [32, L]
    oh_w = (ll % 32 == np.arange(32)[:, None]).astype(np.float32)

    # tables [4(t), 64(c), 63(m)] -> const8 rows 0:64; q-side tables x8
    tables = np.stack([
        np.asarray(rel_h_q, np.float32)[::-1].T * 8.0,
        np.asarray(rel_w_q, np.float32)[::-1].T * 8.0,
        np.asarray(rel_h_k, np.float32).T,
        np.asarray(rel_w_k, np.float32).T,
    ], 0)
    const8 = np.zeros((128, 512), np.float32)
    const8[0:64, 0:252] = tables.transpose(1, 0, 2).reshape(64, 252)
    const8[:, 504:506] = 1.0
    const8 = const8.astype(f8)

    z64 = np.zeros((64, L), np.float32)

    in_maps = []
    for n in range(NCORES):
        q = query[:, n]           # [B, L, C]
        k = key_input[:, n]
        v = value[:, n]
        qT = q.transpose(2, 0, 1)  # [C, B, L]
        kT = k.transpose(2, 0, 1)
        # qf[p]: [128, 2, L]: pair0 = [Q^T/8 ; qterm placeholder]
        #                     pair1 = [onehot_h(q); onehot_w(q); zeros]
        qfa = np.ascontiguousarray(qT.transpose(1, 0, 2) / 8.0).astype(f8)
        qfb1 = np.concatenate([oh_h, oh_w, z64], 0)  # [128, L]
        qfb = np.ascontiguousarray(
            np.broadcast_to(qfb1[None], (B, 128, L))).astype(f8)
        kfa = np.ascontiguousarray(np.concatenate(
            [kT.transpose(1, 0, 2),
             np.broadcast_to(oh_h[None], (B, 32, L)),
             np.broadcast_to(oh_w[None], (B, 32, L))], 1)).astype(f8)
        kfb = np.zeros((B, 64, L), f8)
        # v8[p]: [128, 4(kbp), 2(i), 64]; k = (2*kbp + i)*128 + kp
        v8 = np.ascontiguousarray(
            v.reshape(B, 4, 2, 128, 64).transpose(0, 3, 1, 2, 4)).astype(f8)
        qres = np.ascontiguousarray(
            q.reshape(B, 8, 128, 64).transpose(2, 1, 0, 3)).astype(np.float32)
        in_maps.append(dict(qfa=qfa, qfb=qfb, kfa=kfa, kfb=kfb,
                            const8=const8, v8=v8, qres=qres))

    res = run_bass_kernel_spmd(
        nc, in_maps, core_ids=list(range(NCORES)),
        trace=bool(int(os.environ.get("KERNEL_TRACE", "0"))),
    )
    _CACHED["last_result"] = res

    # outt: [B, 128, 8, 64] -> out[b, n, qb*128+qp, c]
    out = np.stack([r["outt"] for r in res.results], axis=1)  # [B, NH, 128, 8, 64]
    out = out.transpose(0, 1, 3, 2, 4).reshape(B, NH, L, C)
    return np.ascontiguousarray(out).astype(np.float32)


# revision 4
# speedup vs baseline: 1.0208x; 1.0055x over previous
"""Trainium2 Bass kernel v2: decomposed rel-pos attention via fp8 DoubleRow.

Score factorization (per batch p, head n), computed transposed S^T[k, q] in
ONE fp8 DoubleRow matmul (256-row contraction, 0.5 cycles/row):
      pair0 rows 0:64   K^T[c,k]       x  Q^T[c,q]/8
      pair0 rows 64:96  onehot_h(k)    x  qterm_h^T[j,q]
      pair0 rows 96:128 onehot_w(k)    x  qterm_w^T[j,q]
      pair1 rows 0:32   kterm_h^T[j,k] x  onehot_h(q)
      pair1 rows 32:64  kterm_w^T[j,k] x  onehot_w(q)
      pair1 rows 64:128 zeros
    et = exp(S^T - 4)        (ACT engine is the bottleneck at ~34us;
                              -4 keeps fp8 range safe, cancels in softmax)
    out^T[q, c] = (et^T V) / (et^T 1) + Q    (PV emitted in [q, c] layout:
                              65-wide free dim makes it nearly free on PE;
                              denominator lands per-partition so normalize
                              is reciprocal + 2 vector ops, no transposes)

qterm/kterm come from 32-wide fp8 table matmuls grouped by h/w of q/k into
one [128, 1024] PSUM tile (4 partition strips: qh, qw, kh, kw), then two
copies (DVE q-side, Pool k-side) drop them into the operand tiles.
q-side tables are pre-scaled x8 so the matmuls read Q/8 in place.

PSUM start_tensor_calc zeroes lazily per (partition-range x 2KB bank): only
the first matmul touching a bank (per partition strip) carries start=True;
w-strips straddle both banks so zero-writing armer matmuls arm them.

Per-batch operand tiles keep DMA/copy/read dependencies exact, and S
matmuls are emitted one k-pair ahead of PV so the in-order PE queue never
head-of-line blocks the exp pipeline.

Sharding: head-parallel across the 8 NeuronCores (4 batches x 1 head each).
"""

import os
import sys

import numpy as np

if "/opt/trn_rl_repo" not in sys.path:
    sys.path.insert(0, "/opt/trn_rl_repo")

B, NH, L, C = 4, 8, 1024, 64
NCORES = 8

_CACHED = {}


def _build_nc():
    import concourse.bass as bass  # noqa: F401
    import concourse.tile as tile
    from concourse import bacc, mybir

    f32 = mybir.dt.float32
    fp8 = mybir.dt.float8e4
    Exp = mybir.ActivationFunctionType.Exp
    DR = mybir.MatmulPerfMode.DoubleRow

    nc = bacc.Bacc("TRN2", target_bir_lowering=False, debug=False)

    qfa_d = nc.dram_tensor("qfa", [B, 64, L], fp8, kind="ExternalInput")
    qfb_d = nc.dram_tensor("qfb", [B, 128, L], fp8, kind="ExternalInput")
    kfa_d = nc.dram_tensor("kfa", [B, 128, L], fp8, kind="ExternalInput")
    kfb_d = nc.dram_tensor("kfb", [B, 64, L], fp8, kind="ExternalInput")
    const8_d = nc.dram_tensor("const8", [128, 512], fp8, kind="ExternalInput")
    v8_d = nc.dram_tensor("v8", [B, 128, 4, 2, 64], fp8, kind="ExternalInput")
    qres_d = nc.dram_tensor("qres", [128, 8, B, 64], f32, kind="ExternalInput")
    outt = nc.dram_tensor("outt", [B, 128, 8, 64], f32, kind="ExternalOutput")

    with tile.TileContext(nc) as tc:
        with (
            tc.tile_pool(name="persist", bufs=1) as persist,
            tc.tile_pool(name="work", bufs=2) as work,
            tc.tile_pool(name="expp", bufs=3) as expp,
            tc.tile_pool(name="outp", bufs=2) as outp,
        ):
            biasc = persist.tile([128, 1], f32)
            nc.gpsimd.memset(biasc, -4.0)
            z64a = persist.tile([64, 32], fp8)
            nc.gpsimd.memset(z64a, 0.0)
            warm = persist.tile([128, 1], f32)
            nc.scalar.activation(warm, biasc, Exp, bias=biasc)

            const_t = persist.tile([128, 512], fp8)
            nc.sync.dma_start(const_t, const8_d[:])
            tbl = const_t[0:64, 0:252].rearrange("c (t m) -> c t m", t=4)
            ones2 = const_t[:, 504:506].rearrange("p (i o) -> p i o", o=1)

            # per-batch operand tiles; batch-0 pieces issued first
            Qf = [persist.tile([128, 2, L], fp8, name=f"Qf{p}") for p in range(B)]
            Kf = [persist.tile([128, 2, L], fp8, name=f"Kf{p}") for p in range(B)]
            v8t = [persist.tile([128, 4, 2, 64], fp8, name=f"v8{p}") for p in range(B)]
            def load_batch(p, eng):
                # aug regions Qf[64:128, 0] / Kf[0:64, 1] are device-written;
                # batch 0's aug inputs ride SWDGE to dodge the serial HWDGE
                eng.dma_start(Qf[p][0:64, 0, :], qfa_d[p])
                eng.dma_start(Kf[p][:, 0, :], kfa_d[p])
                beng = nc.scalar if p == 0 else nc.sync
                beng.dma_start(Qf[p][:, 1, :], qfb_d[p])
                beng.dma_start(Kf[p][64:128, 1, :], kfb_d[p])
                nc.scalar.dma_start(v8t[p], v8_d[p])

            load_batch(0, nc.gpsimd)
            for p in range(1, B):
                load_batch(p, nc.sync)
            qres_t = persist.tile([128, 8, B, 64], f32)
            nc.sync.dma_start(qres_t, qres_d[:])

            # prologue aug(0) in two separate 2-bank tiles (q-side, k-side):
            # byte-interval dependency tracking is partition-blind, so a
            # shared tile would serialize k-strips behind q-copies; the
            # scoped pool frees its banks before the main pools open
            with tc.tile_pool(name="ps_aug0", bufs=2, space="PSUM") as ps_aug0:
                tq0 = ps_aug0.tile([64, L], f32, name="tq0")
                tk0 = ps_aug0.tile([64, L], f32, name="tk0")
                for t in range(4):
                    tile_, row0 = (tq0, 32 * t) if t < 2 else (tk0, 32 * (t - 2))
                    tile_w = tile_.rearrange("j (h w) -> j w h", w=32)
                    src = (Qf[0] if t < 2 else Kf[0])[0:64, 0, :]
                    src_w = src.rearrange("c (h w) -> c w h", w=32)
                    if t % 2 == 1:
                        for bank in range(2):
                            nc.tensor.matmul(
                                tile_[row0:row0 + 32,
                                      512 * bank:512 * bank + 1],
                                z64a, const_t[0:64, 0:1],
                                start=True, stop=True,
                                tile_position=(0, row0),
                                skip_group_check=True)
                    for g in range(32):
                        lhsT = tbl[:, t, 31 - g:63 - g]
                        if t % 2 == 0:
                            rhs = src[:, 32 * g:32 * g + 32]
                            out = tile_[row0:row0 + 32, 32 * g:32 * g + 32]
                            st = g in (0, 16)
                        else:
                            rhs = src_w[:, g, :]
                            out = tile_w[row0:row0 + 32, g, :]
                            st = False
                        nc.tensor.matmul(out, lhsT, rhs, start=st, stop=True,
                                         tile_position=(0, row0),
                                         skip_group_check=True)
                    if t == 1:
                        nc.vector.tensor_copy(Qf[0][64:128, 0, 0:512],
                                              tq0[:, 0:512])
                        nc.vector.tensor_copy(Qf[0][64:128, 0, 512:L],
                                              tq0[:, 512:L])
                    elif t == 3:
                        nc.vector.tensor_copy(Kf[0][0:64, 1, 0:512],
                                              tk0[:, 0:512])
                        nc.vector.tensor_copy(Kf[0][0:64, 1, 512:L],
                                              tk0[:, 512:L])

            with (
                tc.tile_pool(name="ps_aug", bufs=1, space="PSUM") as ps_aug,
                tc.tile_pool(name="ps_s", bufs=2, space="PSUM") as ps_s,
                tc.tile_pool(name="ps_pv", bufs=1, space="PSUM") as ps_pv,
                tc.tile_pool(name="ps_den", bufs=1, space="PSUM") as ps_den,
            ):
                def emit_aug_strips(ps_a, ps_aw, p, ts):
                    for t in ts:
                        src = (Qf[p] if t < 2 else Kf[p])[0:64, 0, :]
                        src_w = src.rearrange("c (h w) -> c w h", w=32)
                        if t % 2 == 1:
                            for bank in range(2):
                                nc.tensor.matmul(
                                    ps_a[32 * t:32 * t + 32,
                                         512 * bank:512 * bank + 1],
                                    z64a, const_t[0:64, 0:1],
                                    start=True, stop=True,
                                    tile_position=(0, 32 * t),
                                    skip_group_check=True)
                        for g in range(32):
                            lhsT = tbl[:, t, 31 - g:63 - g]
                            if t % 2 == 0:
                                rhs = src[:, 32 * g:32 * g + 32]
                                out = ps_a[32 * t:32 * t + 32, 32 * g:32 * g + 32]
                                st = g in (0, 16)
                            else:
                                rhs = src_w[:, g, :]
                                out = ps_aw[32 * t:32 * t + 32, g, :]
                                st = False
                            nc.tensor.matmul(out, lhsT, rhs, start=st, stop=True,
                                             tile_position=(0, 32 * t),
                                             skip_group_check=True)

                aug_tiles = {}

                def emit_aug_step(p, step, act_assist=False):
                    if step == 0:
                        ps_a = ps_aug.tile([128, L], f32, tag="psa", name="psa")
                        aug_tiles[p] = (ps_a, ps_a.rearrange("j (h w) -> j w h",
                                                             w=32))
                    ps_a, ps_aw = aug_tiles[p]
                    emit_aug_strips(ps_a, ps_aw, p, (step,))
                    # ACT is idle before the first exp, so the prologue
                    # parallelizes the PSUM->SBUF copies across DVE + ACT
                    if step == 1:
                        nc.vector.tensor_copy(Qf[p][64:128, 0, 0:512],
                                              ps_a[0:64, 0:512])
                        (nc.scalar.copy if act_assist
                         else nc.vector.tensor_copy)(
                            Qf[p][64:128, 0, 512:L], ps_a[0:64, 512:L])
                    elif step == 3:
                        nc.vector.tensor_copy(Kf[p][0:64, 1, 0:128],
                                              ps_a[64:128, 0:128])
                        (nc.scalar.copy if act_assist
                         else nc.vector.tensor_copy)(
                            Kf[p][0:64, 1, 128:L], ps_a[64:128, 128:L])

                def emit_aug(p):
                    for step in range(4):
                        emit_aug_step(p, step)

                state = {}

                def emit_s_exp(p, kb):
                    st = state[p]
                    if kb % 2 == 0:
                        st["et2"].append(expp.tile([128, 2, L], fp8,
                                                   tag="et2", name="et2"))
                    et2 = st["et2"][kb // 2]
                    sp = ps_s.tile([128, L], f32, tag="sp", name="sp")
                    for ch in range(2):
                        cs = slice(512 * ch, 512 * (ch + 1))
                        nc.tensor.matmul(
                            sp[:, cs],
                            Kf[p][:, :, 128 * kb:128 * (kb + 1)],
                            Qf[p][:, :, cs],
                            start=True, stop=True, perf_mode=DR)
                    if p == B - 1 and kb == 7:
                        # split the very last exp so the first PV/normalize
                        # half overlaps the second half-exp (shorter drain)
                        for ch in range(2):
                            cs = slice(512 * ch, 512 * (ch + 1))
                            nc.scalar.activation(et2[:, 1, cs], sp[:, cs],
                                                 Exp, bias=biasc)
                    else:
                        nc.scalar.activation(et2[:, kb % 2, :], sp, Exp,
                                             bias=biasc)

                def emit_pv(p, kbp):
                    st = state[p]
                    et2 = st["et2"][kbp]
                    for qb in range(8):
                        lhsT = et2[:, :, 128 * qb:128 * (qb + 1)]
                        first = kbp == 0 and qb == 0
                        nc.tensor.matmul(st["pvt"][:, qb, :], lhsT,
                                         v8t[p][:, kbp, :, :],
                                         start=first, stop=(kbp == 3),
                                         perf_mode=DR, skip_group_check=True)
                        nc.tensor.matmul(st["den"][:, qb:qb + 1], lhsT, ones2,
                                         start=first, stop=(kbp == 3),
                                         perf_mode=DR, skip_group_check=True)

                def emit_norm(p, quarters=False):
                    # normalize + residual in qb-chunks so each output DMA
                    # overlaps the next chunk's vector work
                    st = state[p]
                    r = work.tile([128, 8], f32, tag="r", name="r")
                    nc.vector.reciprocal(r, st["den"])
                    ot = outp.tile([128, 8, 64], f32, tag="ot", name="ot")
                    nch = 4 if quarters else 2
                    w_ = 8 // nch
                    for hb in range(nch):
                        hs = slice(w_ * hb, w_ * hb + w_)
                        otm = work.tile([128, w_, 64], f32, tag="otm",
                                        name="otm", bufs=2)
                        nc.vector.tensor_mul(
                            otm, st["pvt"][:, hs, :],
                            r[:, hs, None].to_broadcast((128, w_, 64)))
                        (nc.gpsimd if hb % 2 else nc.vector).tensor_add(
                            ot[:, hs, :], otm, qres_t[:, hs, p, :])
                        nc.sync.dma_start(outt[p, :, hs, :], ot[:, hs, :])

                # software pipeline: PV(p, m) emits three S/exp slots
                # after exp(p, 2m+1) and the next batch's aug strips spread
                # over kb 2..5, so the in-order PE queue never blocks the
                # exp stream; normalize(p) slides into batch p+1
                stream = [(p, kb) for p in range(B) for kb in range(8)]
                for idx, (p, kb) in enumerate(stream):
                    if kb == 0:
                        state[p] = dict(
                            pvt=ps_pv.tile([128, 8, 64], f32, tag="pvt",
                                           name="pvt"),
                            den=ps_den.tile([128, 8], f32, tag="den",
                                            name="den"),
                            et2=[])
                    emit_s_exp(p, kb)
                    aug0 = 2
                    if aug0 <= kb <= aug0 + 3 and p + 1 < B:
                        emit_aug_step(p + 1, kb - aug0)
                    due = idx - 3
                    if due >= 0:
                        dp, dkb = stream[due]
                        if dkb % 2 == 1:
                            emit_pv(dp, dkb // 2)
                            if dkb == 7:
                                emit_norm(dp)
                # flush: PV2 then the last k-pair + normalize in
                # q-halves pipelined against the split final exp
                lp = B - 1
                emit_pv(lp, 2)
                st = state[lp]
                et2 = st["et2"][3]
                r = work.tile([128, 8], f32, tag="r", name="r")
                ot = outp.tile([128, 8, 64], f32, tag="ot", name="ot")
                for half in range(2):
                    for qb in range(4 * half, 4 * half + 4):
                        lhsT = et2[:, :, 128 * qb:128 * (qb + 1)]
                        nc.tensor.matmul(st["pvt"][:, qb, :], lhsT,
                                         v8t[lp][:, 3, :, :],
                                         start=False, stop=True,
                                         perf_mode=DR, skip_group_check=True)
                        nc.tensor.matmul(st["den"][:, qb:qb + 1], lhsT, ones2,
                                         start=False, stop=True,
                                         perf_mode=DR, skip_group_check=True)
                    hs4 = slice(4 * half, 4 * half + 4)
                    nc.vector.reciprocal(r[:, hs4], st["den"][:, hs4])
                    for sub in range(2):
                        hs = slice(4 * half + 2 * sub, 4 * half + 2 * sub + 2)
                        otm = work.tile([128, 2, 64], f32, tag="otm",
                                        name="otm", bufs=2)
                        nc.vector.tensor_mul(
                            otm, st["pvt"][:, hs, :],
                            r[:, hs, None].to_broadcast((128, 2, 64)))
                        (nc.gpsimd if sub else nc.vector).tensor_add(
                            ot[:, hs, :], otm, qres_t[:, hs, lp, :])
                    nc.sync.dma_start(outt[lp, :, hs4, :], ot[:, hs4, :])

    nc.compile()
    return nc


def _split_c(x):
    # [64, ...] -> [32, 2, ...] with c = 32*i + ci
    return np.ascontiguousarray(
        x.reshape(2, 32, *x.shape[1:]).transpose(1, 0, *range(2, x.ndim + 1)))


def kernel(query, key_input, value, rel_h_q, rel_w_q, rel_h_k, rel_w_k):
    import ml_dtypes
    from concourse.bass_utils import run_bass_kernel_spmd

    f8 = ml_dtypes.float8_e4m3
    query = np.asarray(query, np.float32)
    key_input = np.asarray(key_input, np.float32)
    value = np.asarray(value, np.float32)

    if "nc" not in _CACHED:
        _CACHED["nc"] = _build_nc()
    nc = _CACHED["nc"]

    ll = np.arange(L)
    oh_h = (ll // 32 == np.arange(32)[:, None]).astype(np.float32)  # [32, L]
    oh_w = (ll % 32 == np.arange(32)[:, None]).astype(np.float32)

    # tables [4(t), 64(c), 63(m)] -> const8 rows 0:64; q-side tables x8
    tables = np.stack([
        np.asarray(rel_h_q, np.float32)[::-1].T * 8.0,
        np.asarray(rel_w_q, np.float32)[::-1].T * 8.0,
        np.asarray(rel_h_k, np.float32).T,
        np.asarray(rel_w_k, np.float32).T,
    ], 0)
    const8 = np.zeros((128, 512), np.float32)
    const8[0:64, 0:252] = tables.transpose(1, 0, 2).reshape(64, 252)
    const8[:, 504:506] = 1.0
    const8 = const8.astype(f8)

    z64 = np.zeros((64, L), np.float32)

    in_maps = []
    for n in range(NCORES):
        q = query[:, n]           # [B, L, C]
        k = key_input[:, n]
        v = value[:, n]
        qT = q.transpose(2, 0, 1)  # [C, B, L]
        kT = k.transpose(2, 0, 1)
        # qf[p]: [128, 2, L]: pair0 = [Q^T/8 ; qterm placeholder]
        #                     pair1 = [onehot_h(q); onehot_w(q); zeros]
        qfa = np.ascontiguousarray(qT.transpose(1, 0, 2) / 8.0).astype(f8)
        qfb1 = np.concatenate([oh_h, oh_w, z64], 0)  # [128, L]
        qfb = np.ascontiguousarray(
            np.broadcast_to(qfb1[None], (B, 128, L))).astype(f8)
        kfa = np.ascontiguousarray(np.concatenate(
            [kT.transpose(1, 0, 2),
             np.broadcast_to(oh_h[None], (B, 32, L)),
             np.broadcast_to(oh_w[None], (B, 32, L))], 1)).astype(f8)
        kfb = np.zeros((B, 64, L), f8)
        # v8[p]: [128, 4(kbp), 2(i), 64]; k = (2*kbp + i)*128 + kp
        v8 = np.ascontiguousarray(
            v.reshape(B, 4, 2, 128, 64).transpose(0, 3, 1, 2, 4)).astype(f8)
        qres = np.ascontiguousarray(
            q.reshape(B, 8, 128, 64).transpose(2, 1, 0, 3)).astype(np.float32)
        in_maps.append(dict(qfa=qfa, qfb=qfb, kfa=kfa, kfb=kfb,
                            const8=const8, v8=v8, qres=qres))

    res = run_bass_kernel_spmd(
        nc, in_maps, core_ids=list(range(NCORES)),
        trace=bool(int(os.environ.get("KERNEL_TRACE", "0"))),
    )
    _CACHED["last_result"] = res

    # outt: [B, 128, 8, 64] -> out[b, n, qb*128+qp, c]
    out = np.stack([r["outt"] for r in res.results], axis=1)  # [B, NH, 128, 8, 64]
    out = out.transpose(0, 1, 3, 2, 4).reshape(B, NH, L, C)
    return np.ascontiguousarray(out).astype(np.float32)


# revision 5
# speedup vs baseline: 1.0337x; 1.0126x over previous
"""Trainium2 Bass kernel v2: decomposed rel-pos attention via fp8 DoubleRow.

Score factorization (per batch p, head n), computed transposed S^T[k, q] in
ONE fp8 DoubleRow matmul (256-row contraction, 0.5 cycles/row):
      pair0 rows 0:64   K^T[c,k]       x  Q^T[c,q]/8
      pair0 rows 64:96  onehot_h(k)    x  qterm_h^T[j,q]
      pair0 rows 96:128 onehot_w(k)    x  qterm_w^T[j,q]
      pair1 rows 0:32   kterm_h^T[j,k] x  onehot_h(q)
      pair1 rows 32:64  kterm_w^T[j,k] x  onehot_w(q)
      pair1 rows 64:128 zeros
    et = exp(S^T - 4)        (ACT engine is the bottleneck at ~34us;
                              -4 keeps fp8 range safe, cancels in softmax)
    out^T[q, c] = (et^T V) / (et^T 1) + Q    (PV emitted in [q, c] layout:
                              65-wide free dim makes it nearly free on PE;
                              denominator lands per-partition so normalize
                              is reciprocal + 2 vector ops, no transposes)

qterm/kterm come from 32-wide fp8 table matmuls grouped by h/w of q/k into
one [128, 1024] PSUM tile (4 partition strips: qh, qw, kh, kw), then two
copies (DVE q-side, Pool k-side) drop them into the operand tiles.
q-side tables are pre-scaled x8 so the matmuls read Q/8 in place.

PSUM start_tensor_calc zeroes lazily per (partition-range x 2KB bank): only
the first matmul touching a bank (per partition strip) carries start=True;
w-strips straddle both banks so zero-writing armer matmuls arm them.

Per-batch operand tiles keep DMA/copy/read dependencies exact, and S
matmuls are emitted one k-pair ahead of PV so the in-order PE queue never
head-of-line blocks the exp pipeline.

Sharding: head-parallel across the 8 NeuronCores (4 batches x 1 head each).
"""

import os
import sys

import numpy as np

if "/opt/trn_rl_repo" not in sys.path:
    sys.path.insert(0, "/opt/trn_rl_repo")

B, NH, L, C = 4, 8, 1024, 64
NCORES = 8

_CACHED = {}


def _build_nc():
    import concourse.bass as bass  # noqa: F401
    import concourse.tile as tile
    from concourse import bacc, mybir

    f32 = mybir.dt.float32
    fp8 = mybir.dt.float8e4
    Exp = mybir.ActivationFunctionType.Exp
    DR = mybir.MatmulPerfMode.DoubleRow

    nc = bacc.Bacc("TRN2", target_bir_lowering=False, debug=False)

    qfa_d = nc.dram_tensor("qfa", [B, 64, L], fp8, kind="ExternalInput")
    qfb_d = nc.dram_tensor("qfb", [B, 128, L], fp8, kind="ExternalInput")
    kfa_d = nc.dram_tensor("kfa", [B, 128, L], fp8, kind="ExternalInput")
    kfb_d = nc.dram_tensor("kfb", [B, 64, L], fp8, kind="ExternalInput")
    const8_d = nc.dram_tensor("const8", [128, 512], fp8, kind="ExternalInput")
    v8_d = nc.dram_tensor("v8", [B, 128, 4, 2, 64], fp8, kind="ExternalInput")
    qres_d = nc.dram_tensor("qres", [128, 8, B, 64], f32, kind="ExternalInput")
    outt = nc.dram_tensor("outt", [B, 128, 8, 64], f32, kind="ExternalOutput")

    with tile.TileContext(nc) as tc:
        with (
            tc.tile_pool(name="persist", bufs=1) as persist,
            tc.tile_pool(name="work", bufs=2) as work,
            tc.tile_pool(name="expp", bufs=3) as expp,
            tc.tile_pool(name="outp", bufs=2) as outp,
        ):
            biasc = persist.tile([128, 1], f32)
            nc.gpsimd.memset(biasc, -4.0)
            z64a = persist.tile([64, 32], fp8)
            nc.gpsimd.memset(z64a, 0.0)
            warm = persist.tile([128, 1], f32)
            nc.scalar.activation(warm, biasc, Exp, bias=biasc)

            const_t = persist.tile([128, 512], fp8)
            nc.sync.dma_start(const_t, const8_d[:])
            tbl = const_t[0:64, 0:252].rearrange("c (t m) -> c t m", t=4)
            ones2 = const_t[:, 504:506].rearrange("p (i o) -> p i o", o=1)

            # per-batch operand tiles; batch-0 pieces issued first
            Qf = [persist.tile([128, 2, L], fp8, name=f"Qf{p}") for p in range(B)]
            Kf = [persist.tile([128, 2, L], fp8, name=f"Kf{p}") for p in range(B)]
            v8t = [persist.tile([128, 4, 2, 64], fp8, name=f"v8{p}") for p in range(B)]
            def load_batch(p, eng):
                # aug regions Qf[64:128, 0] / Kf[0:64, 1] are device-written;
                # batch 0's aug inputs ride SWDGE to dodge the serial HWDGE
                eng.dma_start(Qf[p][0:64, 0, :], qfa_d[p])
                eng.dma_start(Kf[p][:, 0, :], kfa_d[p])
                beng = nc.scalar if p == 0 else nc.sync
                beng.dma_start(Qf[p][:, 1, :], qfb_d[p])
                beng.dma_start(Kf[p][64:128, 1, :], kfb_d[p])
                nc.scalar.dma_start(v8t[p], v8_d[p])

            load_batch(0, nc.gpsimd)
            for p in range(1, B):
                load_batch(p, nc.sync)
            qres_t = persist.tile([128, 8, B, 64], f32)
            nc.sync.dma_start(qres_t, qres_d[:])

            # prologue aug(0) in two separate 2-bank tiles (q-side, k-side):
            # byte-interval dependency tracking is partition-blind, so a
            # shared tile would serialize k-strips behind q-copies; the
            # scoped pool frees its banks before the main pools open
            with tc.tile_pool(name="ps_aug0", bufs=2, space="PSUM") as ps_aug0:
                tq0 = ps_aug0.tile([64, L], f32, name="tq0")
                tk0 = ps_aug0.tile([64, L], f32, name="tk0")
                for t in range(4):
                    tile_, row0 = (tq0, 32 * t) if t < 2 else (tk0, 32 * (t - 2))
                    tile_w = tile_.rearrange("j (h w) -> j w h", w=32)
                    src = (Qf[0] if t < 2 else Kf[0])[0:64, 0, :]
                    src_w = src.rearrange("c (h w) -> c w h", w=32)
                    if t % 2 == 1:
                        for bank in range(2):
                            nc.tensor.matmul(
                                tile_[row0:row0 + 32,
                                      512 * bank:512 * bank + 1],
                                z64a, const_t[0:64, 0:1],
                                start=True, stop=True,
                                tile_position=(0, row0),
                                skip_group_check=True)
                    for g in range(32):
                        lhsT = tbl[:, t, 31 - g:63 - g]
                        if t % 2 == 0:
                            rhs = src[:, 32 * g:32 * g + 32]
                            out = tile_[row0:row0 + 32, 32 * g:32 * g + 32]
                            st = g in (0, 16)
                        else:
                            rhs = src_w[:, g, :]
                            out = tile_w[row0:row0 + 32, g, :]
                            st = False
                        nc.tensor.matmul(out, lhsT, rhs, start=st, stop=True,
                                         tile_position=(0, row0),
                                         skip_group_check=True)
                    if t == 1:
                        nc.vector.tensor_copy(Qf[0][64:128, 0, 0:512],
                                              tq0[:, 0:512])
                        nc.vector.tensor_copy(Qf[0][64:128, 0, 512:L],
                                              tq0[:, 512:L])
                    elif t == 3:
                        nc.vector.tensor_copy(Kf[0][0:64, 1, 0:512],
                                              tk0[:, 0:512])
                        nc.vector.tensor_copy(Kf[0][0:64, 1, 512:L],
                                              tk0[:, 512:L])

            with (
                tc.tile_pool(name="ps_aug", bufs=1, space="PSUM") as ps_aug,
                tc.tile_pool(name="ps_s", bufs=2, space="PSUM") as ps_s,
                tc.tile_pool(name="ps_pv", bufs=1, space="PSUM") as ps_pv,
                tc.tile_pool(name="ps_den", bufs=1, space="PSUM") as ps_den,
            ):
                def emit_aug_strips(ps_a, ps_aw, p, ts):
                    for t in ts:
                        src = (Qf[p] if t < 2 else Kf[p])[0:64, 0, :]
                        src_w = src.rearrange("c (h w) -> c w h", w=32)
                        if t % 2 == 1:
                            for bank in range(2):
                                nc.tensor.matmul(
                                    ps_a[32 * t:32 * t + 32,
                                         512 * bank:512 * bank + 1],
                                    z64a, const_t[0:64, 0:1],
                                    start=True, stop=True,
                                    tile_position=(0, 32 * t),
                                    skip_group_check=True)
                        for g in range(32):
                            lhsT = tbl[:, t, 31 - g:63 - g]
                            if t % 2 == 0:
                                rhs = src[:, 32 * g:32 * g + 32]
                                out = ps_a[32 * t:32 * t + 32, 32 * g:32 * g + 32]
                                st = g in (0, 16)
                            else:
                                rhs = src_w[:, g, :]
                                out = ps_aw[32 * t:32 * t + 32, g, :]
                                st = False
                            nc.tensor.matmul(out, lhsT, rhs, start=st, stop=True,
                                             tile_position=(0, 32 * t),
                                             skip_group_check=True)

                aug_tiles = {}

                def emit_aug_step(p, step, act_assist=False):
                    if step == 0:
                        ps_a = ps_aug.tile([128, L], f32, tag="psa", name="psa")
                        aug_tiles[p] = (ps_a, ps_a.rearrange("j (h w) -> j w h",
                                                             w=32))
                    ps_a, ps_aw = aug_tiles[p]
                    emit_aug_strips(ps_a, ps_aw, p, (step,))
                    # ACT is idle before the first exp, so the prologue
                    # parallelizes the PSUM->SBUF copies across DVE + ACT
                    if step == 1:
                        nc.vector.tensor_copy(Qf[p][64:128, 0, 0:512],
                                              ps_a[0:64, 0:512])
                        (nc.scalar.copy if act_assist
                         else nc.vector.tensor_copy)(
                            Qf[p][64:128, 0, 512:L], ps_a[0:64, 512:L])
                    elif step == 3:
                        nc.vector.tensor_copy(Kf[p][0:64, 1, 0:128],
                                              ps_a[64:128, 0:128])
                        (nc.scalar.copy if act_assist
                         else nc.vector.tensor_copy)(
                            Kf[p][0:64, 1, 128:L], ps_a[64:128, 128:L])

                def emit_aug(p):
                    for step in range(4):
                        emit_aug_step(p, step)

                state = {}

                def emit_s_exp(p, kb):
                    st = state[p]
                    if kb % 2 == 0:
                        st["et2"].append(expp.tile([128, 2, L], fp8,
                                                   tag="et2", name="et2"))
                    et2 = st["et2"][kb // 2]
                    sp = ps_s.tile([128, L], f32, tag="sp", name="sp")
                    for ch in range(2):
                        cs = slice(512 * ch, 512 * (ch + 1))
                        nc.tensor.matmul(
                            sp[:, cs],
                            Kf[p][:, :, 128 * kb:128 * (kb + 1)],
                            Qf[p][:, :, cs],
                            start=True, stop=True, perf_mode=DR)
                    if p == B - 1 and kb == 7:
                        # split the very last exp so the first PV/normalize
                        # half overlaps the second half-exp (shorter drain)
                        for ch in range(2):
                            cs = slice(512 * ch, 512 * (ch + 1))
                            nc.scalar.activation(et2[:, 1, cs], sp[:, cs],
                                                 Exp, bias=biasc)
                    else:
                        nc.scalar.activation(et2[:, kb % 2, :], sp, Exp,
                                             bias=biasc)

                def emit_pv(p, kbp):
                    st = state[p]
                    et2 = st["et2"][kbp]
                    for qb in range(8):
                        lhsT = et2[:, :, 128 * qb:128 * (qb + 1)]
                        first = kbp == 0 and qb == 0
                        nc.tensor.matmul(st["pvt"][:, qb, :], lhsT,
                                         v8t[p][:, kbp, :, :],
                                         start=first, stop=(kbp == 3),
                                         perf_mode=DR, skip_group_check=True)
                        nc.tensor.matmul(st["den"][:, qb:qb + 1], lhsT, ones2,
                                         start=first, stop=(kbp == 3),
                                         perf_mode=DR, skip_group_check=True)

                def emit_norm(p, quarters=False):
                    # normalize + residual in qb-chunks so each output DMA
                    # overlaps the next chunk's vector work
                    st = state[p]
                    r = work.tile([128, 8], f32, tag="r", name="r")
                    nc.vector.reciprocal(r, st["den"])
                    ot = outp.tile([128, 8, 64], f32, tag="ot", name="ot")
                    nch = 4 if quarters else 2
                    w_ = 8 // nch
                    for hb in range(nch):
                        hs = slice(w_ * hb, w_ * hb + w_)
                        otm = work.tile([128, w_, 64], f32, tag="otm",
                                        name="otm", bufs=2)
                        nc.vector.tensor_mul(
                            otm, st["pvt"][:, hs, :],
                            r[:, hs, None].to_broadcast((128, w_, 64)))
                        (nc.gpsimd if hb % 2 else nc.vector).tensor_add(
                            ot[:, hs, :], otm, qres_t[:, hs, p, :])
                        nc.sync.dma_start(outt[p, :, hs, :], ot[:, hs, :])

                # software pipeline: PV(p, m) emits three S/exp slots
                # after exp(p, 2m+1) and the next batch's aug strips spread
                # over kb 2..5, so the in-order PE queue never blocks the
                # exp stream; normalize(p) slides into batch p+1
                stream = [(p, kb) for p in range(B) for kb in range(8)]
                for idx, (p, kb) in enumerate(stream):
                    if kb == 0:
                        state[p] = dict(
                            pvt=ps_pv.tile([128, 8, 64], f32, tag="pvt",
                                           name="pvt"),
                            den=ps_den.tile([128, 8], f32, tag="den",
                                            name="den"),
                            et2=[])
                    emit_s_exp(p, kb)
                    aug0 = 2
                    if aug0 <= kb <= aug0 + 3 and p + 1 < B:
                        emit_aug_step(p + 1, kb - aug0)
                    due = idx - 3
                    if due >= 0:
                        dp, dkb = stream[due]
                        if dkb % 2 == 1:
                            emit_pv(dp, dkb // 2)
                            if dkb == 7:
                                emit_norm(dp)
                # flush: PV2 then the last k-pair + normalize in
                # q-halves pipelined against the split final exp
                lp = B - 1
                emit_pv(lp, 2)
                st = state[lp]
                et2 = st["et2"][3]
                r = work.tile([128, 8], f32, tag="r", name="r")
                ot = outp.tile([128, 8, 64], f32, tag="ot", name="ot")
                for half in range(2):
                    for qb in range(4 * half, 4 * half + 4):
                        lhsT = et2[:, :, 128 * qb:128 * (qb + 1)]
                        nc.tensor.matmul(st["pvt"][:, qb, :], lhsT,
                                         v8t[lp][:, 3, :, :],
                                         start=False, stop=True,
                                         perf_mode=DR, skip_group_check=True)
                        nc.tensor.matmul(st["den"][:, qb:qb + 1], lhsT, ones2,
                                         start=False, stop=True,
                                         perf_mode=DR, skip_group_check=True)
                    hs4 = slice(4 * half, 4 * half + 4)
                    nc.vector.reciprocal(r[:, hs4], st["den"][:, hs4])
                    for sub in range(2):
                        hs = slice(4 * half + 2 * sub, 4 * half + 2 * sub + 2)
                        otm = work.tile([128, 2, 64], f32, tag="otm",
                                        name="otm", bufs=2)
                        nc.vector.tensor_mul(
                            otm, st["pvt"][:, hs, :],
                            r[:, hs, None].to_broadcast((128, 2, 64)))
                        # first sub's add on Pool overlaps the second sub's
                        # DVE mul; the critical last add stays on fast DVE
                        (nc.vector if sub else nc.gpsimd).tensor_add(
                            ot[:, hs, :], otm, qres_t[:, hs, lp, :])
                    nc.sync.dma_start(outt[lp, :, hs4, :], ot[:, hs4, :])

    nc.compile()
    return nc


def _split_c(x):
    # [64, ...] -> [32, 2, ...] with c = 32*i + ci
    return np.ascontiguousarray(
        x.reshape(2, 32, *x.shape[1:]).transpose(1, 0, *range(2, x.ndim + 1)))


def kernel(query, key_input, value, rel_h_q, rel_w_q, rel_h_k, rel_w_k):
    import ml_dtypes
    from concourse.bass_utils import run_bass_kernel_spmd

    f8 = ml_dtypes.float8_e4m3
    query = np.asarray(query, np.float32)
    key_input = np.asarray(key_input, np.float32)
    value = np.asarray(value, np.float32)

    if "nc" not in _CACHED:
        _CACHED["nc"] = _build_nc()
    nc = _CACHED["nc"]

    ll = np.arange(L)
    oh_h = (ll // 32 == np.arange(32)[:, None]).astype(np.float32)  # [32, L]
    oh_w = (ll % 32 == np.arange(32)[:, None]).astype(np.float32)

    # tables [4(t), 64(c), 63(m)] -> const8 rows 0:64; q-side tables x8
    tables = np.stack([
        np.asarray(rel_h_q, np.float32)[::-1].T * 8.0,
        np.asarray(rel_w_q, np.float32)[::-1].T * 8.0,
        np.asarray(rel_h_k, np.float32).T,
        np.asarray(rel_w_k, np.float32).T,
    ], 0)
    const8 = np.zeros((128, 512), np.float32)
    const8[0:64, 0:252] = tables.transpose(1, 0, 2).reshape(64, 252)
    const8[:, 504:506] = 1.0
    const8 = const8.astype(f8)

    z64 = np.zeros((64, L), np.float32)

    in_maps = []
    for n in range(NCORES):
        q = query[:, n]           # [B, L, C]
        k = key_input[:, n]
        v = value[:, n]
        qT = q.transpose(2, 0, 1)  # [C, B, L]
        kT = k.transpose(2, 0, 1)
        # qf[p]: [128, 2, L]: pair0 = [Q^T/8 ; qterm placeholder]
        #                     pair1 = [onehot_h(q); onehot_w(q); zeros]
        qfa = np.ascontiguousarray(qT.transpose(1, 0, 2) / 8.0).astype(f8)
        qfb1 = np.concatenate([oh_h, oh_w, z64], 0)  # [128, L]
        qfb = np.ascontiguousarray(
            np.broadcast_to(qfb1[None], (B, 128, L))).astype(f8)
        kfa = np.ascontiguousarray(np.concatenate(
            [kT.transpose(1, 0, 2),
             np.broadcast_to(oh_h[None], (B, 32, L)),
             np.broadcast_to(oh_w[None], (B, 32, L))], 1)).astype(f8)
        kfb = np.zeros((B, 64, L), f8)
        # v8[p]: [128, 4(kbp), 2(i), 64]; k = (2*kbp + i)*128 + kp
        v8 = np.ascontiguousarray(
            v.reshape(B, 4, 2, 128, 64).transpose(0, 3, 1, 2, 4)).astype(f8)
        qres = np.ascontiguousarray(
            q.reshape(B, 8, 128, 64).transpose(2, 1, 0, 3)).astype(np.float32)
        in_maps.append(dict(qfa=qfa, qfb=qfb, kfa=kfa, kfb=kfb,
                            const8=const8, v8=v8, qres=qres))

    res = run_bass_kernel_spmd(
        nc, in_maps, core_ids=list(range(NCORES)),
        trace=bool(int(os.environ.get("KERNEL_TRACE", "0"))),
    )
    _CACHED["last_result"] = res

    # outt: [B, 128, 8, 64] -> out[b, n, qb*128+qp, c]
    out = np.stack([r["outt"] for r in res.results], axis=1)  # [B, NH, 128, 8, 64]
    out = out.transpose(0, 1, 3, 2, 4).reshape(B, NH, L, C)
    return np.ascontiguousarray(out).astype(np.float32)
